# revision 6
# baseline (speedup 1.0000x reference)
"""MultiLabelSoftMarginLoss (logits=True path) on 8 Trainium2 NeuronCores.

Math (per sample b, C classes, K labels t_bk, ls = log_sigmoid):
  pos_mean_b = (1/K) sum_k ls(g_bk),  g_bk = x[b, t_bk]
  neg_mean_b = [sum_c ls(-x_bc) - sum_{unique labels u} ls(-x_bu)] / (C - n_unique_b)
  loss = -mean_b(pos_mean_b + neg_mean_b)

Bulk term: sum_c ls(-x_c) = ln prod_c sigmoid(-x_c). Each chunk computes
s = sigmoid(-x) on the ACT engine (bf16 out), multiplies groups of 16
together with four unit-stride in-place fold multiplies on the DVE
(2x bf16 mode), and deferred Ln+row-accumulate passes recover the sum
while touching only 1/16 of the elements. (sigma products of 16 stay far
above bf16 subnormal range for randn inputs; bf16 product noise is
zero-mean and averages out over 50257*2048 terms.)

Positive/dedup correction: gather g via per-column indirect DMAs; then
ls(g) = g + ln sigmoid(-g) on tiny [128, K] tiles. First-occurrence dedup
weights and the 1/(C - n_unique) denominators derive purely from the int32
label indices, so they are precomputed on the host alongside the flat
gather offsets and streamed in as tiny inputs.

Schedule: the x stream (DMA -> sigmoid -> folds) owns the Sync/ACT/DVE
queues end to end. The chunk DMA issues are gated on the ACT completion
counter bufs-deep, so xpool is kept 13 deep to absorb the injected Ln
sessions without stalling the issue queue. Ln work runs as three pinned
ACT sessions: A (all of blk0) early in blk1's stream, B1 (most of blk1)
a few chunks before the end, and a ~0.5us B2 tail. The odd-width
remainder chunks sit mid-block so the stream ends on full aligned chunks.
Tiny index/weight loads and the output stores ride the GpSimd queue.

Data-parallel: 2048 rows sharded 256/core; host sums 8x256 per-row
losses and negates.
"""

import numpy as np

import concourse.bacc as bacc
import concourse.bass as bass
import concourse.mybir as mybir
import concourse.tile as tile
from concourse.bass_utils import run_bass_kernel_spmd
from concourse.tile_rust import add_dep_helper

B, C, K = 2048, 50257, 20
NCORES = 8
RPC = B // NCORES  # rows per core
P = 128
NBLK = RPC // P  # row blocks of 128 partitions per core
CHUNK = 3072
REM = 1105  # odd remainder width; padded to 1120 for the 4 fold-halvings

# blk0 leads with three 1024-col chunks so ACT starts early; blk1 ends
# with three so the stream tail drains fast. Both blocks carry the odd
# remainder mid-block so the stream never ends on an unaligned transfer.
WIDTHS0 = [1024] * 3 + [CHUNK] * 7 + [REM] + [CHUNK] * 8
WIDTHS1 = [CHUNK] * 8 + [REM] + [CHUNK] * 7 + [1024] * 3
assert sum(WIDTHS0) == C and sum(WIDTHS1) == C
BLK_WIDTHS = [WIDTHS0, WIDTHS1]


def _pc(w):
    return (-(-w // 16) * 16) // 16


PROD_COLS = sum(_pc(w) for w in WIDTHS0)  # 3142

SESSION_A_CI = 2  # blk0's Ln session anchors after blk1 chunk 2
SESSION_B1_CI = 10  # blk1's main Ln session anchors after blk1 chunk 10
B1_COLS = sum(_pc(w) for w in WIDTHS1[: SESSION_B1_CI + 1])  # 1990

F32 = mybir.dt.float32
BF16 = mybir.dt.bfloat16
I32 = mybir.dt.int32
AF = mybir.ActivationFunctionType
ALU = mybir.AluOpType
AX = mybir.AxisListType

_CACHE = {}


def _fold_products(nc, s, width, pt_all, pt_off):
    """Reduce s[:, :width] (bf16) to width/16 group products written to
    pt_all[:, pt_off : pt_off + width//16] via four unit-stride in-place
    folds. Returns the last fold instruction (vector-queue anchor)."""
    w2, w4, w8, w16 = width // 2, width // 4, width // 8, width // 16
    nc.vector.tensor_tensor(
        out=s[:, :w2], in0=s[:, :w2], in1=s[:, w2:width], op=ALU.mult
    )
    nc.vector.tensor_tensor(
        out=s[:, :w4], in0=s[:, :w4], in1=s[:, w4:w2], op=ALU.mult
    )
    nc.vector.tensor_tensor(
        out=s[:, :w8], in0=s[:, :w8], in1=s[:, w8:w4], op=ALU.mult
    )
    return nc.vector.tensor_tensor(
        out=pt_all[:, pt_off : pt_off + w16],
        in0=s[:, :w16],
        in1=s[:, w16:w8],
        op=ALU.mult,
    )


def _build():
    nc = bacc.Bacc(
        "TRN2", target_bir_lowering=False, debug=False, num_devices=NCORES,
        num_swdge_queues=4,
    )
    x = nc.dram_tensor("x", [RPC, C], F32, kind="ExternalInput").ap()
    o = nc.dram_tensor("o", [RPC, K], I32, kind="ExternalInput").ap()
    w = nc.dram_tensor("w", [RPC, K], F32, kind="ExternalInput").ap()
    rd = nc.dram_tensor("rd", [RPC, 1], F32, kind="ExternalInput").ap()
    out = nc.dram_tensor("out", [NBLK, P], F32, kind="ExternalOutput").ap()

    with tile.TileContext(nc) as tc:
        with (
            tc.tile_pool(name="xpool", bufs=13) as xpool,
            tc.tile_pool(name="scr", bufs=3) as scr,
            tc.tile_pool(name="small", bufs=2) as small,
        ):
            # ---- tiny inputs on the GpSimd queue (Sync stays stream-only) ----
            offts, wts, rdts, gs = [], [], [], []
            for blk in range(NBLK):
                rows = slice(blk * P, (blk + 1) * P)
                offt = small.tile([P, K], I32, tag="offt")
                nc.gpsimd.dma_start(out=offt[:], in_=o[rows, :])
                offts.append(offt)
                wt = small.tile([P, K], F32, tag="wt")
                nc.gpsimd.dma_start(out=wt[:], in_=w[rows, :])
                wts.append(wt)
                rdt = small.tile([P, 1], F32, tag="rdt")
                nc.gpsimd.dma_start(out=rdt[:], in_=rd[rows, :])
                rdts.append(rdt)

            # ---- gathers: 2*K per-column indirect DMAs on GpSimd, spread
            # over the 4 SWDGE queues; done by ~55us, consumed from ~75us ----
            for blk in range(NBLK):
                g = small.tile([P, K], F32, tag="g")
                gs.append(g)
                for k in range(K):
                    inst = nc.gpsimd.indirect_dma_start(
                        out=g[:, k : k + 1],
                        out_offset=None,
                        in_=x[:, :],
                        in_offset=bass.IndirectOffsetOnAxis(
                            ap=offts[blk][:, k : k + 1], axis=1
                        ),
                    )
                    qi = (blk * K + k) % 4
                    if qi:
                        inst.ins.queue = f"qPoolDynamic{qi}"

            # ---- per-block persistent tiles ----
            pt_alls, sgns, lnsgns, lnsgn_sums = [], [], [], []
            for blk in range(NBLK):
                pt_alls.append(
                    small.tile([P, PROD_COLS], BF16, tag="pt_all",
                               name=f"pt_all{blk}")
                )
                sgns.append(
                    small.tile([P, K], F32, tag="sgn", name=f"sgn{blk}")
                )
                lnsgns.append(
                    small.tile([P, K], F32, tag="lnsgn", name=f"lnsgn{blk}")
                )
                lnsgn_sums.append(
                    small.tile([P, 1], F32, tag="lnsgn_sum",
                               name=f"lnsgn_sum{blk}")
                )
            T0 = small.tile([P, 1], F32, tag="T0")
            T1a = small.tile([P, 1], F32, tag="T1a")
            T1b = small.tile([P, 1], F32, tag="T1b")

            def emit_session(blk, lo, hi, anchor_sig, accT):
                """ACT session: sigma(-g) [sigmoid table] -> Ln(sgn) with
                accum -> Ln(pt cols lo:hi) with accum, pinned contiguously
                after anchor_sig so only one table round-trip happens."""
                sgn_act = nc.scalar.activation(
                    sgns[blk][:], gs[blk][:], AF.Sigmoid, scale=-1.0
                )
                add_dep_helper(
                    sgn_act.ins, anchor_sig.ins, sync=False,
                    reason="session sigma after anchor sigmoid",
                )
                ln_small = nc.scalar.activation(
                    lnsgns[blk][:], sgns[blk][:], AF.Ln,
                    accum_out=lnsgn_sums[blk][:],
                )
                add_dep_helper(
                    ln_small.ins, sgn_act.ins, sync=False,
                    reason="session Ln small after sigma",
                )
                ln_big = nc.scalar.activation(
                    pt_alls[blk][:, lo:hi], pt_alls[blk][:, lo:hi],
                    AF.Ln, accum_out=accT[:],
                )
                add_dep_helper(
                    ln_big.ins, ln_small.ins, sync=False,
                    reason="contiguous Ln session",
                )
                return ln_big

            def emit_epilogue(blk, T_tile, anchor_fold):
                """DVE combine for one block; chain head pinned after
                anchor_fold so the vector queue never blocks mid-stream."""
                gsum = small.tile([P, 1], F32, tag="gsum")
                r1 = nc.vector.reduce_sum(out=gsum[:], in_=gs[blk][:], axis=AX.X)
                if anchor_fold is not None:
                    add_dep_helper(
                        r1.ins, anchor_fold.ins, sync=False,
                        reason="epilogue after stream fold",
                    )
                wl = small.tile([P, K], F32, tag="wl")
                nc.vector.tensor_tensor(
                    out=wl[:], in0=wts[blk][:], in1=lnsgns[blk][:], op=ALU.mult
                )
                dsum = small.tile([P, 1], F32, tag="dsum")
                nc.vector.reduce_sum(out=dsum[:], in_=wl[:], axis=AX.X)
                # neg_mean = (T - dsum) * (1/(C-U));  T = sum_c ls(-x_c)
                negm = small.tile([P, 1], F32, tag="negm")
                nc.vector.tensor_sub(out=negm[:], in0=T_tile[:], in1=dsum[:])
                nc.vector.tensor_mul(out=negm[:], in0=negm[:], in1=rdts[blk][:])
                # pos_mean = (sum_k g_k + sum_k ls(-g_k)) / K
                posm = small.tile([P, 1], F32, tag="posm")
                nc.vector.tensor_add(
                    out=posm[:], in0=gsum[:], in1=lnsgn_sums[blk][:]
                )
                nc.vector.tensor_scalar(
                    out=posm[:], in0=posm[:], scalar1=1.0 / K, scalar2=None,
                    op0=ALU.mult,
                )
                loss = small.tile([P, 1], F32, tag="loss")
                nc.vector.tensor_add(out=loss[:], in0=posm[:], in1=negm[:])
                # on the Sync queue: the GpSimd queue's teardown DRAIN is
                # ~9us, and it runs after that queue's LAST instruction --
                # keeping outputs off GpSimd hides the drain mid-stream
                nc.sync.dma_start(out=out[blk, :, None], in_=loss[:])

            # ---- the stream ----
            prev_sig = None
            pending = None  # ACT work the next sigmoid must wait for
            epi0_due = False
            for blk in range(NBLK):
                rows = slice(blk * P, (blk + 1) * P)
                pt_all = pt_alls[blk]
                c0 = 0
                pt_off = 0
                for ci, cw in enumerate(BLK_WIDTHS[blk]):
                    cwp = -(-cw // 16) * 16
                    xt = xpool.tile([P, CHUNK], F32, tag="xt")
                    if cw != cwp:
                        # pad -> sigmoid(30)=1.0 -> neutral for products
                        nc.vector.memset(xt[:, cw:cwp], -30.0)
                    nc.sync.dma_start(out=xt[:, :cw], in_=x[rows, c0 : c0 + cw])
                    s = scr.tile([P, CHUNK], BF16, tag="s")
                    sig = nc.scalar.activation(
                        s[:, :cwp], xt[:, :cwp], AF.Sigmoid, scale=-1.0
                    )
                    anchor = pending or prev_sig
                    if anchor is not None:
                        add_dep_helper(
                            sig.ins, anchor.ins, sync=False,
                            reason="pin ACT stream order",
                        )
                    pending = None
                    prev_sig = sig
                    last_fold = _fold_products(nc, s, cwp, pt_all, pt_off)
                    c0 += cw
                    pt_off += cwp // 16

                    if blk == 1 and ci == SESSION_A_CI:
                        pending = emit_session(0, 0, PROD_COLS, sig, T0)
                        epi0_due = True
                    elif blk == 1 and ci == SESSION_A_CI + 1 and epi0_due:
                        # blk0's DVE epilogue one chunk later: T0 is ready
                        # before this chunk's folds finish, so no DVE stall
                        emit_epilogue(0, T0, last_fold)
                        epi0_due = False
                    elif blk == 1 and ci == SESSION_B1_CI:
                        pending = emit_session(1, 0, B1_COLS, sig, T1a)

            # ---- tail: blk1's remaining product columns + combine ----
            ln_b2 = nc.scalar.activation(
                pt_alls[1][:, B1_COLS:], pt_alls[1][:, B1_COLS:], AF.Ln,
                accum_out=T1b[:],
            )
            add_dep_helper(
                ln_b2.ins, prev_sig.ins, sync=False, reason="tail Ln"
            )
            nc.vector.tensor_add(out=T1a[:], in0=T1a[:], in1=T1b[:])
            emit_epilogue(1, T1a, None)

    nc.compile()
    return nc


def kernel(inputs: np.ndarray, targets: np.ndarray, _trace: bool = False):
    inputs = np.ascontiguousarray(inputs, dtype=np.float32)
    targets = np.ascontiguousarray(targets, dtype=np.int32)
    assert inputs.shape == (B, C) and targets.shape == (B, K)

    if "nc" not in _CACHE:
        _CACHE["nc"] = _build()
    nc = _CACHE["nc"]

    # index preprocessing on the host: flat gather offsets, first-occurrence
    # dedup weights, and the masked-count reciprocals (all from int32 labels)
    t64 = targets.astype(np.int64)
    offs_np = (t64 + (np.arange(B, dtype=np.int64) % RPC)[:, None] * C).astype(
        np.int32
    )
    first = t64[:, :, None] == t64[:, None, :]  # [B, K, K]
    # w_bk = 1 iff no earlier equal label in the row
    dup = np.tril(first, k=-1).any(axis=2)
    w_np = (~dup).astype(np.float32)
    u_np = w_np.sum(axis=1)
    rd_np = (1.0 / (C - u_np)).astype(np.float32)[:, None]

    in_maps = [
        {
            "x": inputs[i * RPC : (i + 1) * RPC],
            "o": offs_np[i * RPC : (i + 1) * RPC],
            "w": w_np[i * RPC : (i + 1) * RPC],
            "rd": rd_np[i * RPC : (i + 1) * RPC],
        }
        for i in range(NCORES)
    ]
    res = run_bass_kernel_spmd(
        nc, in_maps, core_ids=list(range(NCORES)), trace=_trace
    )
    _CACHE["last_results"] = res

    per_row = np.concatenate(
        [res.results[i]["out"].reshape(-1) for i in range(NCORES)]
    )
    return np.float32(-np.mean(per_row, dtype=np.float64))


# revision 10
# speedup vs baseline: 1.1131x; 1.1131x over previous
"""MultiLabelSoftMarginLoss (logits=True path) on 8 Trainium2 NeuronCores.

Math (per sample b, C classes, K labels t_bk, ls = log_sigmoid):
  pos_mean_b = (1/K) sum_k ls(g_bk),  g_bk = x[b, t_bk]
  neg_mean_b = [sum_c ls(-x_bc) - sum_{unique labels u} ls(-x_bu)] / (C - n_unique_b)
  loss = -mean_b(pos_mean_b + neg_mean_b)

Bulk term: sum_c ls(-x_c) = ln prod_c sigmoid(-x_c). Each chunk computes
s = sigmoid(-x) on the ACT engine (bf16 out), multiplies groups of 16
together with four unit-stride in-place fold multiplies on the DVE
(2x bf16 mode), and deferred Ln+row-accumulate passes recover the sum
while touching only 1/16 of the elements. (sigma products of 16 stay far
above bf16 subnormal range for randn inputs; bf16 product noise is
zero-mean and averages out over 50257*2048 terms.)

Positive/dedup correction: gather g via per-column indirect DMAs; then
ls(g) = g + ln sigmoid(-g) on tiny [128, K] tiles. First-occurrence dedup
weights and the 1/(C - n_unique) denominators derive purely from the int32
label indices, so they are precomputed on the host alongside the flat
gather offsets and streamed in as tiny inputs.

Schedule: the x stream (DMA -> sigmoid -> folds) owns the Sync/ACT/DVE
queues end to end. The chunk DMA issues are gated on the ACT completion
counter bufs-deep, so xpool is kept 13 deep to absorb the injected Ln
sessions without stalling the issue queue. Ln work runs as three pinned
ACT sessions: A (all of blk0) early in blk1's stream, B1 (most of blk1)
a few chunks before the end, and a ~0.5us B2 tail. The odd-width
remainder chunks sit mid-block so the stream ends on full aligned chunks.
Tiny index/weight loads and the output stores ride the GpSimd queue.

Data-parallel: 2048 rows sharded 256/core; host sums 8x256 per-row
losses and negates.
"""

import numpy as np

import concourse.bacc as bacc
import concourse.bass as bass
import concourse.mybir as mybir
import concourse.tile as tile
from concourse.bass_utils import run_bass_kernel_spmd
from concourse.tile_rust import add_dep_helper

B, C, K = 2048, 50257, 20
NCORES = 8
RPC = B // NCORES  # rows per core
P = 128
NBLK = RPC // P  # row blocks of 128 partitions per core
CHUNK = 3072
REM = 1105  # odd remainder width; padded to 1120 for the 4 fold-halvings

# blk0 leads with three 1024-col chunks so ACT starts early; blk1 ends
# with three so the stream tail drains fast. Both blocks carry the odd
# remainder mid-block so the stream never ends on an unaligned transfer.
WIDTHS0 = [1024] * 3 + [CHUNK] * 7 + [REM] + [CHUNK] * 8
WIDTHS1 = [CHUNK] * 8 + [REM] + [CHUNK] * 7 + [1024] * 3
assert sum(WIDTHS0) == C and sum(WIDTHS1) == C
BLK_WIDTHS = [WIDTHS0, WIDTHS1]


def _pc(w):
    return (-(-w // 16) * 16) // 16


PROD_COLS = sum(_pc(w) for w in WIDTHS0)  # 3142

SESSION_A_CI = 2  # blk0's Ln session anchors after blk1 chunk 2
SESSION_B1_CI = 10  # blk1's main Ln session anchors after blk1 chunk 10
B1_COLS = sum(_pc(w) for w in WIDTHS1[: SESSION_B1_CI + 1])  # 1990

F32 = mybir.dt.float32
BF16 = mybir.dt.bfloat16
I32 = mybir.dt.int32
AF = mybir.ActivationFunctionType
ALU = mybir.AluOpType
AX = mybir.AxisListType

_CACHE = {}


def _fold_products(nc, s, width, pt_all, pt_off):
    """Reduce s[:, :width] (bf16) to width/16 group products written to
    pt_all[:, pt_off : pt_off + width//16] via four unit-stride in-place
    folds. Returns the last fold instruction (vector-queue anchor)."""
    w2, w4, w8, w16 = width // 2, width // 4, width // 8, width // 16
    nc.vector.tensor_tensor(
        out=s[:, :w2], in0=s[:, :w2], in1=s[:, w2:width], op=ALU.mult
    )
    nc.vector.tensor_tensor(
        out=s[:, :w4], in0=s[:, :w4], in1=s[:, w4:w2], op=ALU.mult
    )
    nc.vector.tensor_tensor(
        out=s[:, :w8], in0=s[:, :w8], in1=s[:, w8:w4], op=ALU.mult
    )
    return nc.vector.tensor_tensor(
        out=pt_all[:, pt_off : pt_off + w16],
        in0=s[:, :w16],
        in1=s[:, w16:w8],
        op=ALU.mult,
    )


def _build():
    nc = bacc.Bacc(
        "TRN2", target_bir_lowering=False, debug=False, num_devices=NCORES,
        num_swdge_queues=4,
    )
    x = nc.dram_tensor("x", [RPC, C], F32, kind="ExternalInput").ap()
    o = nc.dram_tensor("o", [RPC, K], I32, kind="ExternalInput").ap()
    w = nc.dram_tensor("w", [RPC, K], F32, kind="ExternalInput").ap()
    rd = nc.dram_tensor("rd", [RPC, 1], F32, kind="ExternalInput").ap()
    out = nc.dram_tensor("out", [NBLK, P], F32, kind="ExternalOutput").ap()

    with tile.TileContext(nc) as tc:
        with (
            tc.tile_pool(name="xpool", bufs=13) as xpool,
            tc.tile_pool(name="scr", bufs=3) as scr,
            tc.tile_pool(name="small", bufs=2) as small,
        ):
            # ---- tiny inputs on the GpSimd queue (Sync stays stream-only) ----
            offts, wts, rdts, gs = [], [], [], []
            for blk in range(NBLK):
                rows = slice(blk * P, (blk + 1) * P)
                offt = small.tile([P, K], I32, tag="offt")
                nc.gpsimd.dma_start(out=offt[:], in_=o[rows, :])
                offts.append(offt)
                wt = small.tile([P, K], F32, tag="wt")
                nc.gpsimd.dma_start(out=wt[:], in_=w[rows, :])
                wts.append(wt)
                rdt = small.tile([P, 1], F32, tag="rdt")
                nc.gpsimd.dma_start(out=rdt[:], in_=rd[rows, :])
                rdts.append(rdt)

            # ---- gathers: 2*K per-column indirect DMAs on GpSimd, spread
            # over the 4 SWDGE queues; done by ~55us, consumed from ~75us ----
            for blk in range(NBLK):
                g = small.tile([P, K], F32, tag="g")
                gs.append(g)
                for k in range(K):
                    inst = nc.gpsimd.indirect_dma_start(
                        out=g[:, k : k + 1],
                        out_offset=None,
                        in_=x[:, :],
                        in_offset=bass.IndirectOffsetOnAxis(
                            ap=offts[blk][:, k : k + 1], axis=1
                        ),
                    )
                    qi = (blk * K + k) % 4
                    if qi:
                        inst.ins.queue = f"qPoolDynamic{qi}"

            # ---- per-block persistent tiles ----
            pt_alls, sgns, lnsgns, lnsgn_sums = [], [], [], []
            for blk in range(NBLK):
                pt_alls.append(
                    small.tile([P, PROD_COLS], BF16, tag="pt_all",
                               name=f"pt_all{blk}")
                )
                sgns.append(
                    small.tile([P, K], F32, tag="sgn", name=f"sgn{blk}")
                )
                lnsgns.append(
                    small.tile([P, K], F32, tag="lnsgn", name=f"lnsgn{blk}")
                )
                lnsgn_sums.append(
                    small.tile([P, 1], F32, tag="lnsgn_sum",
                               name=f"lnsgn_sum{blk}")
                )
            T0 = small.tile([P, 1], F32, tag="T0")
            T1a = small.tile([P, 1], F32, tag="T1a")
            T1b = small.tile([P, 1], F32, tag="T1b")

            def emit_session(blk, lo, hi, anchor_sig, accT):
                """ACT session: sigma(-g) [sigmoid table] -> Ln(sgn) with
                accum -> Ln(pt cols lo:hi) with accum, pinned contiguously
                after anchor_sig so only one table round-trip happens."""
                sgn_act = nc.scalar.activation(
                    sgns[blk][:], gs[blk][:], AF.Sigmoid, scale=-1.0
                )
                add_dep_helper(
                    sgn_act.ins, anchor_sig.ins, sync=False,
                    reason="session sigma after anchor sigmoid",
                )
                ln_small = nc.scalar.activation(
                    lnsgns[blk][:], sgns[blk][:], AF.Ln,
                    accum_out=lnsgn_sums[blk][:],
                )
                add_dep_helper(
                    ln_small.ins, sgn_act.ins, sync=False,
                    reason="session Ln small after sigma",
                )
                ln_big = nc.scalar.activation(
                    pt_alls[blk][:, lo:hi], pt_alls[blk][:, lo:hi],
                    AF.Ln, accum_out=accT[:],
                )
                add_dep_helper(
                    ln_big.ins, ln_small.ins, sync=False,
                    reason="contiguous Ln session",
                )
                return ln_big

            def emit_epilogue(blk, T_tile, anchor_fold, out_engine):
                """DVE combine for one block; chain head pinned after
                anchor_fold so the vector queue never blocks mid-stream.
                out_engine picks the DMA queue for the result store: the
                in-order Sync queue must NEVER carry it mid-stream (head-
                of-line blocks later chunk issues); GpSimd is safe mid-
                stream (its big teardown DRAIN then still hides under the
                stream) and Scalar is safe at the very end."""
                gsum = small.tile([P, 1], F32, tag="gsum")
                r1 = nc.vector.reduce_sum(out=gsum[:], in_=gs[blk][:], axis=AX.X)
                if anchor_fold is not None:
                    add_dep_helper(
                        r1.ins, anchor_fold.ins, sync=False,
                        reason="epilogue after stream fold",
                    )
                wl = small.tile([P, K], F32, tag="wl")
                nc.vector.tensor_tensor(
                    out=wl[:], in0=wts[blk][:], in1=lnsgns[blk][:], op=ALU.mult
                )
                dsum = small.tile([P, 1], F32, tag="dsum")
                nc.vector.reduce_sum(out=dsum[:], in_=wl[:], axis=AX.X)
                # neg_mean = (T - dsum) * (1/(C-U));  T = sum_c ls(-x_c)
                negm = small.tile([P, 1], F32, tag="negm")
                nc.vector.tensor_sub(out=negm[:], in0=T_tile[:], in1=dsum[:])
                nc.vector.tensor_mul(out=negm[:], in0=negm[:], in1=rdts[blk][:])
                # pos_mean = (sum_k g_k + sum_k ls(-g_k)) / K
                posm = small.tile([P, 1], F32, tag="posm")
                nc.vector.tensor_add(
                    out=posm[:], in0=gsum[:], in1=lnsgn_sums[blk][:]
                )
                nc.vector.tensor_scalar(
                    out=posm[:], in0=posm[:], scalar1=1.0 / K, scalar2=None,
                    op0=ALU.mult,
                )
                loss = small.tile([P, 1], F32, tag="loss")
                nc.vector.tensor_add(out=loss[:], in0=posm[:], in1=negm[:])
                out_engine.dma_start(out=out[blk, :, None], in_=loss[:])

            # ---- the stream ----
            prev_sig = None
            pending = None  # ACT work the next sigmoid must wait for
            epi0_due = False
            for blk in range(NBLK):
                rows = slice(blk * P, (blk + 1) * P)
                pt_all = pt_alls[blk]
                c0 = 0
                pt_off = 0
                for ci, cw in enumerate(BLK_WIDTHS[blk]):
                    cwp = -(-cw // 16) * 16
                    xt = xpool.tile([P, CHUNK], F32, tag="xt")
                    if cw != cwp:
                        # pad -> sigmoid(30)=1.0 -> neutral for products
                        nc.vector.memset(xt[:, cw:cwp], -30.0)
                    nc.sync.dma_start(out=xt[:, :cw], in_=x[rows, c0 : c0 + cw])
                    s = scr.tile([P, CHUNK], BF16, tag="s")
                    sig = nc.scalar.activation(
                        s[:, :cwp], xt[:, :cwp], AF.Sigmoid, scale=-1.0
                    )
                    anchor = pending or prev_sig
                    if anchor is not None:
                        add_dep_helper(
                            sig.ins, anchor.ins, sync=False,
                            reason="pin ACT stream order",
                        )
                    pending = None
                    prev_sig = sig
                    last_fold = _fold_products(nc, s, cwp, pt_all, pt_off)
                    c0 += cw
                    pt_off += cwp // 16

                    if blk == 1 and ci == SESSION_A_CI:
                        pending = emit_session(0, 0, PROD_COLS, sig, T0)
                        epi0_due = True
                    elif blk == 1 and ci == SESSION_A_CI + 1 and epi0_due:
                        # blk0's DVE epilogue one chunk later: T0 is ready
                        # before this chunk's folds finish, so no DVE stall
                        emit_epilogue(0, T0, last_fold, nc.gpsimd)
                        epi0_due = False
                    elif blk == 1 and ci == SESSION_B1_CI:
                        pending = emit_session(1, 0, B1_COLS, sig, T1a)

            # ---- tail: blk1's remaining product columns + combine ----
            ln_b2 = nc.scalar.activation(
                pt_alls[1][:, B1_COLS:], pt_alls[1][:, B1_COLS:], AF.Ln,
                accum_out=T1b[:],
            )
            add_dep_helper(
                ln_b2.ins, prev_sig.ins, sync=False, reason="tail Ln"
            )
            nc.vector.tensor_add(out=T1a[:], in0=T1a[:], in1=T1b[:])
            emit_epilogue(1, T1a, None, nc.scalar)

    nc.compile()
    return nc


def kernel(inputs: np.ndarray, targets: np.ndarray, _trace: bool = False):
    inputs = np.ascontiguousarray(inputs, dtype=np.float32)
    targets = np.ascontiguousarray(targets, dtype=np.int32)
    assert inputs.shape == (B, C) and targets.shape == (B, K)

    if "nc" not in _CACHE:
        _CACHE["nc"] = _build()
    nc = _CACHE["nc"]

    # index preprocessing on the host: flat gather offsets, first-occurrence
    # dedup weights, and the masked-count reciprocals (all from int32 labels)
    t64 = targets.astype(np.int64)
    offs_np = (t64 + (np.arange(B, dtype=np.int64) % RPC)[:, None] * C).astype(
        np.int32
    )
    first = t64[:, :, None] == t64[:, None, :]  # [B, K, K]
    # w_bk = 1 iff no earlier equal label in the row
    dup = np.tril(first, k=-1).any(axis=2)
    w_np = (~dup).astype(np.float32)
    u_np = w_np.sum(axis=1)
    rd_np = (1.0 / (C - u_np)).astype(np.float32)[:, None]

    in_maps = [
        {
            "x": inputs[i * RPC : (i + 1) * RPC],
            "o": offs_np[i * RPC : (i + 1) * RPC],
            "w": w_np[i * RPC : (i + 1) * RPC],
            "rd": rd_np[i * RPC : (i + 1) * RPC],
        }
        for i in range(NCORES)
    ]
    res = run_bass_kernel_spmd(
        nc, in_maps, core_ids=list(range(NCORES)), trace=_trace
    )
    _CACHE["last_results"] = res

    per_row = np.concatenate(
        [res.results[i]["out"].reshape(-1) for i in range(NCORES)]
    )
    return np.float32(-np.mean(per_row, dtype=np.float64))
